# revision 2
# baseline (speedup 1.0000x reference)
"""Multi-head attention (B=2, S=2048, E=1024, H=16, D=64) on 8 Trainium2 cores.

Sharding: data-parallel over batch (2 groups of 4 cores), SEQUENCE-parallel
within each group: each core owns 512 query rows but computes ALL 16 heads
for them. K and V are computed redundantly on every core (+~9 GFLOP of cheap
bf16 matmul) in exchange for ZERO cross-core communication - no collectives,
no cross-core sync, no gather tail, no exposure to collective-latency
variance. Each core's y slice is its own 512 rows of the final output.

Structure (per core, batch b = core//4, row-block r = core%4):
  - prologue: the core's own 512 x rows are transposed (PE, fp32) and
    projected to q^T for all 16 heads (bf16).
  - x streams in 4 super-chunks of 512 rows; each is transposed and
    projected to k^T / v for ALL heads; the head-group-0 attention for the
    just-produced k-chunks runs right behind it, so ScalarE exp starts
    ~20us into the kernel.
  - attention: 4 head-groups (4 heads each) x 16 k-chunks: scores
    (row-packed K=64 pairs, bf16: f32r measures ~351ns/MM vs bf16 133ns)
    -> exp on ScalarE (scale=1/8 folded in; no max-subtraction: logits
    ~N(0,1)) -> PV one k-chunk behind so the strict-FIFO engine queues
    never stall each other.
  - PV + softmax denominator fused in one matmul: lhsT = [v_h0|ones|v_h1]
    per head pair - the ones block is shared by both heads (odd heads read
    [ones|v_h1], so their PSUM rows are denominator-first; the normalize
    slices flip accordingly). fp32 accumulation in 4 persistent PSUM banks,
    range-tracked across head-groups.
  - normalization (reciprocal+mul, DVE) writes out^T bf16; out_proj
    ([512,1024]@[1024,1024] bf16, fp32 accumulate) emits y directly.
  PSUM: 4 banks persistent accumulators + 4 banks shared work pool.
"""

import numpy as np
from contextlib import ExitStack

import concourse.tile as tile
from concourse import bacc, mybir
from concourse.bass_utils import run_bass_kernel_spmd
from concourse.masks import make_identity

B, S, E, H, D = 2, 2048, 1024, 16, 64
N_CORES = 8
QB = 512           # query rows owned per core

F32 = mybir.dt.float32
BF16 = mybir.dt.bfloat16

_cached = None


def build(reps=1, profile=False):
    nc = bacc.Bacc("TRN2", target_bir_lowering=False, debug=False,
                   num_devices=N_CORES)

    xq_d = nc.dram_tensor("xq", [QB, E], F32, kind="ExternalInput").ap()
    x_d = nc.dram_tensor("x", [S, E], F32, kind="ExternalInput").ap()
    wq_d = nc.dram_tensor("wq", [E, E], BF16, kind="ExternalInput").ap()
    wk_d = nc.dram_tensor("wk", [E, E], BF16, kind="ExternalInput").ap()
    wv_d = nc.dram_tensor("wv", [E, E], BF16, kind="ExternalInput").ap()
    wo_d = nc.dram_tensor("wo", [E, E], BF16, kind="ExternalInput").ap()
    y_d = nc.dram_tensor("y", [QB, E], F32, kind="ExternalOutput").ap()

    with tile.TileContext(nc) as tc, ExitStack() as ctx:
        glob = ctx.enter_context(tc.tile_pool(name="glob", bufs=1))
        # persistent PSUM accumulators: one [128, 512] bank per head of the
        # active head-group; range-tracked so group handoff needs no spare
        # banks.
        pvg = ctx.enter_context(tc.tile_pool(name="pvg", bufs=1,
                                             space="PSUM"))
        fz_t = pvg.tile([128, 4, 512], F32, tag="fz")

        qT_t = glob.tile([128, 8, QB], BF16, tag="qT")    # q^T by head pair
        kT_t = glob.tile([128, 8, S], BF16, tag="kT")     # k^T by head pair
        # v_aug by (k-chunk, head-pair): [v_h0 | ones | v_h1] (192 cols)
        v_t = glob.tile([128, 16, 8, 192], BF16, tag="v")
        wo_t = glob.tile([128, 8, E], BF16, tag="wo")
        outT_t = glob.tile([128, 8, QB], BF16, tag="outT")
        ident = glob.tile([128, 128], F32, tag="ident")
        ones_f = glob.tile([128, 64], F32, tag="ones_f")

        make_identity(nc, ident[:])
        nc.gpsimd.memset(ones_f[:], 1.0)
        for sc in range(16):
            for p in range(8):
                nc.vector.tensor_copy(v_t[:, sc, p, 64:128], ones_f[:])
        nc.sync.dma_start(wo_t[:], wo_d.rearrange("(c p) n -> p c n", p=128))

        for _rep in range(reps):
            _emit_body(nc, tc, xq_d, x_d, wq_d, wk_d, wv_d, y_d,
                       qT_t, kT_t, v_t, wo_t, outT_t, ident, fz_t, profile)

    nc.compile()
    return nc


def _emit_body(nc, tc, xq_d, x_d, wq_d, wk_d, wv_d, y_d,
               qT_t, kT_t, v_t, wo_t, outT_t, ident, fz_t, profile=False):
    with ExitStack() as cph:
        # shared 2-buffer PSUM work pool (4 banks): transposes, projections,
        # scores, and the out_proj accumulator all rotate through it.
        wkp = cph.enter_context(tc.tile_pool(name="wkp", bufs=2,
                                             space="PSUM"))
        expp = cph.enter_context(tc.tile_pool(name="expp", bufs=4))
        recp = cph.enter_context(tc.tile_pool(name="recp", bufs=2))

        def ps_tile():
            ps = wkp.tile([128, 1024], F32, tag="ps")
            return ps

        def pv(hg, kc, exs):
            for lp in range(2):              # local pair within the group
                p = 2 * hg + lp
                for par in range(2):
                    # even head reads [v|ones], odd reads [ones|v] (PSUM
                    # rows are denominator-first for odd heads)
                    nc.tensor.matmul(
                        fz_t[:, 2 * lp + par, :],
                        v_t[:, kc, p, 64 * par:64 * par + 128],
                        exs[lp][:, par * 512:(par + 1) * 512],
                        start=(kc == 0), stop=(kc == 15))

        def attn_steps(hg, kcs, prev):
            # scores + exp for each k-chunk; PV one chunk behind so the
            # strict-FIFO PE queue never blocks ScalarE
            for kc in kcs:
                sts, exs = [], []
                for lp in range(2):
                    hp = 2 * hg + lp         # global head-pair index
                    st = ps_tile()
                    for par in range(2):     # row-packed K=64 pair
                        lo, hi = par * 64, (par + 1) * 64
                        nc.tensor.matmul(
                            st[:, par * 512:(par + 1) * 512],
                            kT_t[lo:hi, hp, kc * 128:(kc + 1) * 128],
                            qT_t[lo:hi, hp, :],
                            start=True, stop=True)
                    sts.append(st)
                for lp in range(2):
                    ex = expp.tile([128, 1024], BF16, tag="ex")
                    nc.scalar.activation(ex[:], sts[lp][:],
                                         mybir.ActivationFunctionType.Exp,
                                         scale=0.125)
                    exs.append(ex)
                if prev is not None:
                    pv(*prev)
                prev = (hg, kc, exs)
            return prev

        def normalize(hg):
            for lp in range(2):
                hp = 2 * hg + lp
                for par in range(2):
                    # even heads: rows 0:64 = PV, 64:128 = denominator;
                    # odd heads read [ones|v] so their rows are flipped
                    zlo = 64 * (1 - par)
                    nlo = 64 * par
                    rc = recp.tile([64, 512], F32, tag="rc")
                    nc.vector.reciprocal(
                        rc[:], fz_t[zlo:zlo + 64, 2 * lp + par, :])
                    nc.vector.tensor_mul(
                        outT_t[par * 64:(par + 1) * 64, hp, :],
                        fz_t[nlo:nlo + 64, 2 * lp + par, :], rc[:])

        # ---- prologue: own q rows -> q^T for all heads ----
        with ExitStack() as qph:
            qp = qph.enter_context(tc.tile_pool(name="qp", bufs=1))
            xqload = qph.enter_context(tc.tile_pool(name="xqload", bufs=2))
            xqT_t = qp.tile([128, 8, QB], BF16, tag="xqT")
            wq_t = qp.tile([128, 8, E], BF16, tag="wq")
            for i in range(4):
                xt = xqload.tile([128, E], F32, tag="xq")
                nc.sync.dma_start(xt[:], xq_d[i * 128:(i + 1) * 128, :])
                if i == 0:
                    nc.scalar.dma_start(
                        wq_t[:], wq_d.rearrange("(c p) n -> p c n", p=128))
                for eq in range(2):
                    tp = ps_tile()
                    for j in range(4):
                        ec = eq * 4 + j
                        nc.tensor.transpose(
                            tp[:, j * 128:(j + 1) * 128],
                            xt[:, ec * 128:(ec + 1) * 128], ident[:])
                    nc.vector.tensor_copy(
                        xqT_t[:, eq * 4:(eq + 1) * 4, i * 128:(i + 1) * 128],
                        tp[:, 0:512].rearrange("p (c n) -> p c n", c=4))
            for mc in range(8):
                pp = ps_tile()
                for ec in range(8):
                    nc.tensor.matmul(
                        pp[:, 0:512],
                        wq_t[:, ec, mc * 128:(mc + 1) * 128],
                        xqT_t[:, ec, :],
                        start=(ec == 0), stop=(ec == 7))
                nc.vector.tensor_copy(qT_t[:, mc, :], pp[:, 0:512])

        # ---- k/v for all heads, fused with head-group-0 attention ----
        with ExitStack() as ab:
            abp = ab.enter_context(tc.tile_pool(name="abp", bufs=1))
            xTp = ab.enter_context(tc.tile_pool(name="xTp", bufs=2))
            xload = ab.enter_context(tc.tile_pool(name="xload", bufs=3))
            wk_t = abp.tile([128, 8, E], BF16, tag="wk")
            wv_t = abp.tile([128, 8, E], BF16, tag="wv")

            prev = None
            for g in range(4):       # super-chunk: s rows g*512..g*512+511
                xT_t = xTp.tile([128, 8, 512], BF16, tag="xT")
                for i in range(4):
                    sc = 4 * g + i
                    xt = xload.tile([128, E], F32, tag="x")
                    nc.sync.dma_start(xt[:], x_d[sc * 128:(sc + 1) * 128, :])
                    if sc == 0:
                        nc.scalar.dma_start(
                            wk_t[:],
                            wk_d.rearrange("(c p) n -> p c n", p=128))
                    elif sc == 2:
                        nc.scalar.dma_start(
                            wv_t[:],
                            wv_d.rearrange("(c p) n -> p c n", p=128))
                    for eq in range(2):
                        tp = ps_tile()
                        for j in range(4):
                            ec = eq * 4 + j
                            nc.tensor.transpose(
                                tp[:, j * 128:(j + 1) * 128],
                                xt[:, ec * 128:(ec + 1) * 128], ident[:])
                        nc.vector.tensor_copy(
                            xT_t[:, eq * 4:(eq + 1) * 4,
                                 i * 128:(i + 1) * 128],
                            tp[:, 0:512].rearrange("p (c n) -> p c n", c=4))
                # k^T for all 8 head pairs over this s-block (N=512)
                for mc in range(8):
                    pp = ps_tile()
                    for ec in range(8):
                        nc.tensor.matmul(
                            pp[:, 0:512],
                            wk_t[:, ec, mc * 128:(mc + 1) * 128],
                            xT_t[:, ec, :],
                            start=(ec == 0), stop=(ec == 7))
                    nc.vector.tensor_copy(
                        kT_t[:, mc, g * 512:(g + 1) * 512], pp[:, 0:512])
                # v rows for the 4 chunks of this s-block (all heads)
                for i in range(4):
                    sc = 4 * g + i
                    for nh in range(2):      # two 512-wide column halves
                        pp = ps_tile()
                        for ec in range(8):
                            nc.tensor.matmul(
                                pp[:, 0:512],
                                xT_t[:, ec, i * 128:(i + 1) * 128],
                                wv_t[:, ec, nh * 512:(nh + 1) * 512],
                                start=(ec == 0), stop=(ec == 7))
                        # heads 8nh..8nh+7 -> pairs 4nh..4nh+3; even head
                        # to cols 0:64, odd head to cols 128:192
                        ppv = pp[:, 0:512].rearrange(
                            "p (pr s d) -> p pr s d", pr=4, s=2)
                        for side in range(2):
                            nc.vector.tensor_copy(
                                v_t[:, sc, 4 * nh:4 * nh + 4,
                                    128 * side:128 * side + 64],
                                ppv[:, :, side, :])
                # head-group-0 attention over the k-chunks just produced
                prev = attn_steps(0, range(4 * g, 4 * g + 4), prev)
            pv(*prev)
            normalize(0)

        # ---- remaining head-groups, then out_proj (no communication) ----
        for hg in range(1, 4):
            prev = attn_steps(hg, range(16), None)
            pv(*prev)
            normalize(hg)

        with tc.tile_pool(name="ysb", bufs=2) as ysb:
            for sq in range(4):      # y rows sq*128 .. sq*128+127
                ep = ps_tile()
                for nh in range(2):
                    for hc in range(8):
                        nc.tensor.matmul(
                            ep[:, nh * 512:(nh + 1) * 512],
                            outT_t[:, hc, sq * 128:(sq + 1) * 128],
                            wo_t[:, hc, nh * 512:(nh + 1) * 512],
                            start=(hc == 0), stop=(hc == 7))
                yt = ysb.tile([128, 1024], F32, tag="y")
                nc.vector.tensor_copy(yt[:], ep[:])
                nc.sync.dma_start(y_d[sq * 128:(sq + 1) * 128, :], yt[:])


def _get_nc():
    global _cached
    if _cached is None:
        _cached = build()
    return _cached


def make_in_maps(x, w_qkv, w_out):
    import ml_dtypes
    bf16 = ml_dtypes.bfloat16
    x = np.asarray(x, dtype=np.float32)
    w_qkv = np.asarray(w_qkv, dtype=np.float32)
    w_out = np.asarray(w_out, dtype=np.float32)
    wq = np.ascontiguousarray(w_qkv[:, 0:E]).astype(bf16)
    wk = np.ascontiguousarray(w_qkv[:, E:2 * E]).astype(bf16)
    wv = np.ascontiguousarray(w_qkv[:, 2 * E:3 * E]).astype(bf16)
    wo = np.ascontiguousarray(w_out).astype(bf16)
    in_maps = []
    for c in range(N_CORES):
        b, r = c // 4, c % 4
        in_maps.append({
            "xq": np.ascontiguousarray(x[b, r * QB:(r + 1) * QB, :]),
            "x": np.ascontiguousarray(x[b]),
            "wq": wq, "wk": wk, "wv": wv, "wo": wo,
        })
    return in_maps


def assemble(results):
    y = np.empty((B, S, E), dtype=np.float32)
    for c in range(N_CORES):
        b, r = c // 4, c % 4
        y[b, r * QB:(r + 1) * QB, :] = results[c]["y"]
    return y


def kernel(x, w_qkv, w_out):
    nc = _get_nc()
    res = run_bass_kernel_spmd(nc, make_in_maps(x, w_qkv, w_out),
                               list(range(N_CORES)))
    return assemble(res.results)
